# revision 1
# baseline (speedup 1.0000x reference)
"""Trainium2 Bass kernel for nn_AttentionBlock (GroupNorm + 2-head attention + proj + residual).

Full inputs: x (16, 256, 32, 32) f32, gn_w/gn_b (256,), wq/wk/wv/wp (256, 256).
Sharding: pure data-parallel over batch — 16 / 8 cores = 2 batch elements per core.
No collectives; outputs concatenated on host.

Per-core dataflow (per batch element, channels-on-partitions):
  x (256, 1024)  -> GroupNorm(4 groups) via free-dim reduces + PE group-mask matmuls
  xn -> q, k (256, 1024) = Wq/Wk @ xn ;  vT (1024, 256) = xn^T @ Wv^T
  per head h (hd = 128): ST_jt (j, i) = k_h[:, jt]^T q_h  (scores transposed)
                         ET = exp(scale * ST)      (ACT, fused scale)
                         U (c, i)  = sum_jt vT_jt_h^T @ ET_jt   (PSUM accum)
                         D (., i)  = sum_jt ones^T @ ET_jt      (softmax denom, replicated)
                         ao_h = U * (1/D)
  out = Wp @ [ao_0; ao_1] + x
All big matmuls run in bf16 (fp32 PSUM accumulation); GroupNorm stats/chain in
fp32. A bf16 copy of x feeds the GN/xn path so the critical input DMA is half
the bytes; fp32 x is only used for the residual. Dummy bf16 warmup matmuls trip
the PE HAM clock gate to 2.4 GHz before real work arrives. Weights and GN
constants arrive as two const-blob DMAs. Emission order interleaves the two
batch elements so b1's GN/QKV hide under b0's attention.
"""

import numpy as np

import concourse.bass as bass
import concourse.tile as tile
from concourse import bacc, mybir
from concourse.bass_utils import run_bass_kernel_spmd

N_CORES = 8
B = 16
BPC = B // N_CORES  # batch elements per core
C = 256
H = W = 32
N = H * W  # 1024 spatial positions
HEADS = 2
HD = C // HEADS  # 128 head dim
G = 4  # groupnorm groups
GSIZE = C // G  # 64 channels per group
EPS = 1e-5
ATT_SCALE = float((C * HEADS) ** -0.5)
P = 128  # partitions
CT = C // P  # channel tiles (2)
FT = 512  # matmul moving-dim tile (one fp32 PSUM bank)
NT = N // FT  # n tiles per matmul row pass (2)
JT = N // P  # j tiles (8)
NG = GSIZE * N  # elements per (batch, group)

# const blob column offsets; region [0, CB_W) is the bf16 weight blob,
# [CB_W, CB_F) the fp32 GN-const blob.
OFF_W = 0  # 4 weights (q,k,v,p), each CT*C = 512 cols
OFF_ONES = 2048  # 128 cols of 1.0
CB_W = 2176
OFF_GNWB = 2176  # per ct: 2 cols
OFF_GMASK = 2184  # per ct: G cols
OFF_GMT = 2192  # per ct: 128 cols (values live in rows 0..G-1)
OFF_EPS = 2448  # one col: EPS in rows 0..G-1
CB_F = 2452

f32 = mybir.dt.float32
f32r = mybir.dt.float32r
bf16 = mybir.dt.bfloat16
MM_DT = bf16  # dtype of all big-matmul operands
N_WARMUP = 20  # PE warmup matmuls (~5us busy) to trip the HAM clock gate early
AF = mybir.ActivationFunctionType
ALU = mybir.AluOpType
AX = mybir.AxisListType


def build_bass(bpc=BPC):
    nc = bacc.Bacc("TRN2", target_bir_lowering=False, debug=False)

    x_d = nc.dram_tensor("x", [bpc, C, N], f32, kind="ExternalInput").ap()
    xb_d = nc.dram_tensor("xb", [bpc, C, N], bf16, kind="ExternalInput").ap()
    cbw_d = nc.dram_tensor("cbw", [P, CB_W], MM_DT, kind="ExternalInput").ap()
    cbg_d = nc.dram_tensor("cbg", [P, CB_F - CB_W], f32, kind="ExternalInput").ap()
    out_d = nc.dram_tensor("out", [bpc, C, N], f32, kind="ExternalOutput").ap()

    with tile.TileContext(nc) as tc:
        with (
            tc.tile_pool(name="consts", bufs=1) as consts,
            tc.tile_pool(name="xp", bufs=2) as xp,
            tc.tile_pool(name="xnp", bufs=2) as xnp,
            tc.tile_pool(name="qk", bufs=2) as qk,
            tc.tile_pool(name="vp", bufs=2) as vp,
            tc.tile_pool(name="etp", bufs=2) as etp,
            tc.tile_pool(name="sm", bufs=2) as sm,
            tc.tile_pool(name="scr", bufs=2) as scr,
            tc.tile_pool(name="aop", bufs=2) as aop,
            tc.tile_pool(name="op", bufs=2) as op,
            tc.tile_pool(name="pmm", bufs=2, space="PSUM") as pmm,
            tc.tile_pool(name="pacc", bufs=1, space="PSUM") as pacc,
        ):
            # ---- PE warmup: dense dummy matmuls (no input deps) so the HAM
            # clock gate reaches K=8/8 before the real matmuls start.
            wtile = consts.tile([P, FT], bf16, tag="warm")
            nc.gpsimd.memset(wtile[:], 0.0)
            wps = pacc.tile([P, FT], f32, tag="u")
            for _ in range(N_WARMUP):
                nc.tensor.matmul(
                    wps[:], wtile[:, 0:P], wtile[:], start=True, stop=True
                )

            # ---- inputs: GN consts + x tiles spread over several DMA queues
            # (sync/gpsimd/vector issue to different HWDGE queues -> parallel)
            cbg = consts.tile([P, CB_F - CB_W], f32, tag="cbg")
            nc.sync.dma_start(cbg[:], cbg_d[:])
            # bf16 copy of x feeds GN stats + xn (half the critical bytes);
            # fp32 x arrives later, used only for the residual add.
            dma_engs = [nc.sync, nc.gpsimd, nc.scalar]
            xb_all = []
            for b in range(bpc):
                xbs = []
                for ct in range(CT):
                    xt = xp.tile([P, N], bf16, tag=f"xb{ct}")
                    eng = dma_engs[(b * CT + ct) % 3]
                    eng.dma_start(xt[:], xb_d[b, ct * P : (ct + 1) * P, :])
                    xbs.append(xt)
                xb_all.append(xbs)
            xs_all = []
            for b in range(bpc):
                xs = []
                for ct in range(CT):
                    xt = xp.tile([P, N], f32, tag=f"x{ct}")
                    eng = dma_engs[(b * CT + ct) % 3]
                    eng.dma_start(xt[:], x_d[b, ct * P : (ct + 1) * P, :])
                    xs.append(xt)
                xs_all.append(xs)

            cbw = consts.tile([P, CB_W], MM_DT, tag="cbw")
            nc.scalar.dma_start(cbw[:], cbw_d[:])

            def w_ap(i, kt):  # (128, C) lhsT/rhs slice of weight i, k-tile kt
                base = OFF_W + i * (CT * C) + kt * C
                return cbw[:, base : base + C]

            def g_ap(off):
                return off - CB_W

            gw = [
                cbg[:, g_ap(OFF_GNWB) + ct * 2 : g_ap(OFF_GNWB) + (ct + 1) * 2]
                for ct in range(CT)
            ]
            gm = [
                cbg[:, g_ap(OFF_GMASK) + ct * G : g_ap(OFF_GMASK) + (ct + 1) * G]
                for ct in range(CT)
            ]
            gmt = [
                cbg[0:G, g_ap(OFF_GMT) + ct * P : g_ap(OFF_GMT) + (ct + 1) * P]
                for ct in range(CT)
            ]
            ones = cbw[:, OFF_ONES : OFF_ONES + P]
            eps_ap = cbg[0:G, g_ap(OFF_EPS) : g_ap(OFF_EPS) + 1]
            WQ, WK, WV, WP_ = 0, 1, 2, 3

            s12_all = {}

            def gn_stats(b):
                xbs = xb_all[b]
                s12s = []
                for ct in range(CT):
                    s12 = sm.tile([P, 2], f32, tag=f"s12_{ct}")
                    nc.vector.reduce_sum(s12[:, 0:1], xbs[ct][:], AX.X)
                    sq = scr.tile([P, N], f32, tag="sq")
                    nc.scalar.activation(
                        sq[:], xbs[ct][:], AF.Square, accum_out=s12[:, 1:2]
                    )
                    s12s.append(s12)
                s12_all[b] = s12s

            def gn_chain(b):
                """gstats matmul -> rstd/mean -> per-channel scale/bias -> xn."""
                s12s = s12_all[b]
                xbs = xb_all[b]
                gstats = pmm.tile([G, 2], f32, tag="mm")
                for ct in range(CT):
                    nc.tensor.matmul(
                        gstats[:],
                        gm[ct],
                        s12s[ct][:],
                        start=(ct == 0),
                        stop=(ct == CT - 1),
                    )
                # gstats = [mean, ex2] (masks pre-scaled by 1/NG on host)
                mrs = sm.tile([G, 2], f32, tag="mrs")  # col0 = rstd, col1 = mean
                nc.vector.tensor_copy(mrs[:, 1:2], gstats[:, 0:1])
                negvar = sm.tile([G, 1], f32, tag="negvar")
                nc.vector.scalar_tensor_tensor(
                    negvar[:],
                    mrs[:, 1:2],
                    mrs[:, 1:2],
                    gstats[:, 1:2],
                    ALU.mult,
                    ALU.subtract,
                )
                std = sm.tile([G, 1], f32, tag="std")
                nc.scalar.activation(
                    std[:], negvar[:], AF.Sqrt, bias=eps_ap, scale=-1.0
                )
                nc.vector.reciprocal(mrs[:, 0:1], std[:])

                sbias = []
                for ct in range(CT):
                    bc = pmm.tile([P, 2], f32, tag="mm")
                    nc.tensor.matmul(bc[:], gmt[ct], mrs[:], start=True, stop=True)
                    scale = sm.tile([P, 1], f32, tag=f"scale{ct}")
                    nc.vector.tensor_tensor(scale[:], bc[:, 0:1], gw[ct][:, 0:1], ALU.mult)
                    nbias = sm.tile([P, 1], f32, tag=f"nbias{ct}")
                    nc.vector.tensor_tensor(nbias[:], bc[:, 1:2], scale[:], ALU.mult)
                    nc.vector.tensor_tensor(
                        nbias[:], gw[ct][:, 1:2], nbias[:], ALU.subtract
                    )
                    sbias.append((scale, nbias))

                xns = []
                for ct in range(CT):
                    xn = xnp.tile([P, N], MM_DT, tag=f"xn{ct}")
                    for nt in range(NT):
                        nc.vector.tensor_scalar(
                            xn[:, nt * FT : (nt + 1) * FT],
                            xbs[ct][:, nt * FT : (nt + 1) * FT],
                            sbias[ct][0][:],
                            sbias[ct][1][:],
                            ALU.mult,
                            ALU.add,
                        )
                    xns.append(xn)
                return xns

            def qkv(b, xns):
                qs, ks = [], []
                for wi, outl, name in ((WQ, qs, "q"), (WK, ks, "k")):
                    for ot in range(CT):
                        ps = pmm.tile([P, N], f32, tag="mm")
                        for nt in range(NT):
                            for kt in range(CT):
                                nc.tensor.matmul(
                                    ps[:, nt * FT : (nt + 1) * FT],
                                    w_ap(wi, kt)[:, ot * P : (ot + 1) * P],
                                    xns[kt][:, nt * FT : (nt + 1) * FT],
                                    start=(kt == 0),
                                    stop=(kt == CT - 1),
                                )
                        t = qk.tile([P, N], MM_DT, tag=f"{name}{ot}")
                        for nt in range(NT):
                            sl = slice(nt * FT, (nt + 1) * FT)
                            if name == "q" and ot == 0:
                                nc.scalar.copy(t[:, sl], ps[:, sl])
                            else:
                                nc.vector.tensor_copy(t[:, sl], ps[:, sl])
                        outl.append(t)
                vT = vp.tile([P, JT * C], MM_DT, tag="vt")
                for mt in range(JT):
                    ps = pmm.tile([P, C], f32, tag="mm")
                    for kt in range(CT):
                        nc.tensor.matmul(
                            ps[:],
                            xns[kt][:, mt * P : (mt + 1) * P],
                            w_ap(WV, kt),
                            start=(kt == 0),
                            stop=(kt == CT - 1),
                        )
                    nc.vector.tensor_copy(vT[:, mt * C : (mt + 1) * C], ps[:])
                return qs, ks, vT

            def attn(b, qs, ks, vT, filler=None):
                aos = []
                for h in range(HEADS):
                    qh, kh = qs[h], ks[h]
                    et = etp.tile([P, JT * N], MM_DT, tag="et")
                    for jt in range(JT):
                        st = pmm.tile([P, N], f32, tag="mm")
                        for nt in range(NT):
                            nc.tensor.matmul(
                                st[:, nt * FT : (nt + 1) * FT],
                                kh[:, jt * P : (jt + 1) * P],
                                qh[:, nt * FT : (nt + 1) * FT],
                                start=True,
                                stop=True,
                            )
                        nc.scalar.activation(
                            et[:, jt * N : (jt + 1) * N],
                            st[:],
                            AF.Exp,
                            scale=ATT_SCALE,
                        )
                    u = pacc.tile([P, N], f32, tag="u")
                    dd = pacc.tile([P, N], f32, tag="d")
                    for jt in range(JT):
                        if filler is not None and h == HEADS - 1 and jt == JT - 2:
                            filler()
                            filler = None
                        for nt in range(NT):
                            sl = slice(jt * N + nt * FT, jt * N + (nt + 1) * FT)
                            nc.tensor.matmul(
                                dd[:, nt * FT : (nt + 1) * FT],
                                ones,
                                et[:, sl],
                                start=(jt == 0),
                                stop=(jt == JT - 1),
                            )
                            nc.tensor.matmul(
                                u[:, nt * FT : (nt + 1) * FT],
                                vT[:, jt * C + h * HD : jt * C + (h + 1) * HD],
                                et[:, sl],
                                start=(jt == 0),
                                stop=(jt == JT - 1),
                            )
                    r = scr.tile([P, N], f32, tag="r")
                    ao = aop.tile([P, N], MM_DT, tag=f"ao{h}")
                    for nt in range(NT):
                        sl = slice(nt * FT, (nt + 1) * FT)
                        nc.vector.reciprocal_approx_fast(out=r[:, sl], in_=dd[:, sl])
                        nc.vector.tensor_tensor(
                            ao[:, sl], u[:, sl], r[:, sl], ALU.mult
                        )
                    aos.append(ao)
                return aos

            def proj_out(b, aos):
                xs = xs_all[b]
                pss, os_ = [], []
                for ot in range(CT):
                    ps = pmm.tile([P, N], f32, tag="mm")
                    pss.append(ps)
                    o = op.tile([P, N], f32, tag=f"o{ot}")
                    os_.append(o)
                for nt in range(NT):
                    sl = slice(nt * FT, (nt + 1) * FT)
                    for ot in range(CT):
                        for hh in range(HEADS):
                            nc.tensor.matmul(
                                pss[ot][:, sl],
                                w_ap(WP_, hh)[:, ot * P : (ot + 1) * P],
                                aos[hh][:, sl],
                                start=(hh == 0),
                                stop=(hh == HEADS - 1),
                            )
                    for ot in range(CT):
                        nc.vector.tensor_tensor(
                            os_[ot][:, sl], pss[ot][:, sl], xs[ot][:, sl], ALU.add
                        )
                        nc.sync.dma_start(
                            out_d[b, ot * P : (ot + 1) * P, sl], os_[ot][:, sl]
                        )

            # Interleaved schedule: b1's GN runs during b0's QKV/attention,
            # b1's QKV fills PE while b0's softmax epilogue runs on DVE.
            gn_stats(0)
            xns0 = gn_chain(0)
            # bridge burst: keep PE busy (and HAM warm) while DVE finishes xn
            wps2 = pacc.tile([P, FT], f32, tag="d")
            for _ in range(16):
                nc.tensor.matmul(
                    wps2[:], wtile[:, 0:P], wtile[:], start=True, stop=True
                )
            qkv_b0 = qkv(0, xns0)
            if bpc > 1:
                gn_stats(1)
                xns1 = gn_chain(1)
                aos0 = attn(0, *qkv_b0)
                qkv_b1 = qkv(1, xns1)
                proj_out(0, aos0)
                aos1 = attn(1, *qkv_b1)
                proj_out(1, aos1)
            else:
                aos0 = attn(0, *qkv_b0)
                proj_out(0, aos0)

    nc.compile()
    return nc


def build_const_blob(gn_w, gn_b, wq, wk, wv, wp):
    """Returns (cbw bf16 [P, CB_W], cbg f32 [P, CB_F - CB_W])."""
    import ml_dtypes

    cbw = np.zeros((P, CB_W), np.float32)
    for i, wmat in enumerate((wq, wk, wv, wp)):
        wT = np.asarray(wmat, np.float32).T  # (c_in, c_out)
        for kt in range(CT):
            cbw[:, OFF_W + i * CT * C + kt * C : OFF_W + i * CT * C + (kt + 1) * C] = (
                wT[kt * P : (kt + 1) * P, :]
            )
    cbw[:, OFF_ONES : OFF_ONES + P] = 1.0
    cbg = np.zeros((P, CB_F - CB_W), np.float32)
    gb = OFF_GNWB - CB_W
    cbg[:, gb + 0 : gb + 4 : 2] = np.asarray(gn_w, np.float32).reshape(CT, P).T
    cbg[:, gb + 1 : gb + 4 : 2] = np.asarray(gn_b, np.float32).reshape(CT, P).T
    for ct in range(CT):
        for p in range(P):
            g = (ct * P + p) // GSIZE
            cbg[p, OFF_GMASK - CB_W + ct * G + g] = 1.0 / NG
            cbg[g, OFF_GMT - CB_W + ct * P + p] = 1.0
    cbg[0:G, OFF_EPS - CB_W] = EPS
    return cbw.astype(ml_dtypes.bfloat16), cbg


_NC_CACHE = {}


def kernel(x, gn_w, gn_b, wq, wk, wv, wp):
    x = np.ascontiguousarray(np.asarray(x, dtype=np.float32))
    b, c, h, w = x.shape
    xr = x.reshape(b, c, h * w)
    cbw, cbg = build_const_blob(gn_w, gn_b, wq, wk, wv, wp)

    if "nc" not in _NC_CACHE:
        _NC_CACHE["nc"] = build_bass()
    nc = _NC_CACHE["nc"]

    import ml_dtypes

    xrb = xr.astype(ml_dtypes.bfloat16)
    in_maps = [
        dict(
            x=np.ascontiguousarray(xr[i * BPC : (i + 1) * BPC]),
            xb=np.ascontiguousarray(xrb[i * BPC : (i + 1) * BPC]),
            cbw=cbw,
            cbg=cbg,
        )
        for i in range(N_CORES)
    ]
    res = run_bass_kernel_spmd(nc, in_maps, list(range(N_CORES)))
    out = np.concatenate([res.results[i]["out"] for i in range(N_CORES)], axis=0)
    return out.reshape(b, c, h, w).astype(np.float32)


if __name__ == "__main__":
    rng = np.random.default_rng(0)
    ins = {
        "x": rng.standard_normal((B, C, H, W), dtype=np.float32),
        "gn_w": np.ones((C,), np.float32),
        "gn_b": np.zeros((C,), np.float32),
        "wq": rng.standard_normal((C, C), dtype=np.float32) * C**-0.5,
        "wk": rng.standard_normal((C, C), dtype=np.float32) * C**-0.5,
        "wv": rng.standard_normal((C, C), dtype=np.float32) * C**-0.5,
        "wp": rng.standard_normal((C, C), dtype=np.float32) * C**-0.5,
    }
    out = kernel(**ins)
    print(out.shape, out.dtype)



# revision 11
# speedup vs baseline: 1.0979x; 1.0979x over previous
"""Trainium2 Bass kernel for nn_AttentionBlock (GroupNorm + 2-head attention + proj + residual).

Full inputs: x (16, 256, 32, 32) f32, gn_w/gn_b (256,), wq/wk/wv/wp (256, 256).
Sharding: pure data-parallel over batch - 16 / 8 cores = 2 batch elements per core.
No collectives; outputs concatenated on host.

Per-core dataflow (per batch element, channels-on-partitions):
  xb (256, 1024) bf16 -> GroupNorm stats on DVE (reduce + tensor_tensor_reduce),
  group combine via tiny PE matmuls, xn = xb*scale+bias fused on ACT (per-partition
  affine). q,k = Wq/Wk @ xn (1024-col bf16 MMs); vT tiles = xn_mt^T @ Wv.
  Attention per head: st_jt (j=128, i=1024) = k_jt^T q (one 1024-col MM);
  et = exp(scale*st) on ACT; U (hd, i) and D (softmax denom, replicated) accumulate
  over jt in PSUM as (128, 512) half-tiles; ao = U * recip(D) on DVE.
  proj: out_psum = Wp_h0^T ao0 + Wp_h1^T ao1 + I^T xb  (residual folded in as an
  identity matmul on the bf16 copy of x), DMA'd to DRAM straight from PSUM.

Scheduling: emission order = per-engine queue order. The script software-pipelines:
scores of head n weave instruction-by-instruction with U/D matmuls of head n-1 and
QKV matmuls of the next batch, so the PE never waits on ACT's exp. A short warmup
(7 cold + 8 bridge MMs) trips the HAM clock gate and abuts the first real matmul.

PSUM budget (8 banks): st (128,1024)x2bufs = 4, u+d (128,512) = 2, qm (128,512)x2 = 2.
"""

import numpy as np

import concourse.bass as bass
import concourse.tile as tile
from concourse import bacc, mybir
from concourse.bass_utils import run_bass_kernel_spmd

N_CORES = 8
B = 16
BPC = B // N_CORES  # batch elements per core
C = 256
H = W = 32
N = H * W  # 1024 spatial positions
HEADS = 2
HD = C // HEADS  # 128 head dim
G = 4  # groupnorm groups
GSIZE = C // G  # 64 channels per group
EPS = 1e-5
ATT_SCALE = float((C * HEADS) ** -0.5)
P = 128  # partitions
CT = C // P  # channel tiles (2)
FT = 512  # u/d half-tile free dim
JT = N // P  # j tiles (8)
NG = GSIZE * N  # elements per (batch, group)

# const blob column offsets; [0, CB_W) bf16 weight blob, [0, CB_G) fp32 GN blob.
OFF_W = 0  # 4 weights (q,k,v,p), each CT*C = 512 cols
OFF_ONES = 2048  # 128 cols of 1.0
OFF_ID = 2176  # 128x128 identity
CB_W = 2304
OFF_GNWB = 0  # per ct: 2 cols (gn_w, gn_b)
OFF_GMASK = 4  # per ct: G cols (group mask / NG)
OFF_GMT = 12  # per ct: 128 cols (mask^T, values in rows 0..G-1)
OFF_EPS = 268  # one col: EPS in rows 0..G-1
CB_G = 269

f32 = mybir.dt.float32
bf16 = mybir.dt.bfloat16
MM_DT = bf16
N_WARM1 = 7  # cold warmup MMs before the GN matmuls
N_WARM2 = 8  # bridge MMs between GN matmuls and first QKV matmul
AF = mybir.ActivationFunctionType
ALU = mybir.AluOpType
AX = mybir.AxisListType


def build_bass(bpc=BPC):
    nc = bacc.Bacc("TRN2", target_bir_lowering=False, debug=False)

    xb_d = nc.dram_tensor("xb", [bpc, C, N], bf16, kind="ExternalInput").ap()
    cbw_d = nc.dram_tensor("cbw", [P, CB_W], MM_DT, kind="ExternalInput").ap()
    cbg_d = nc.dram_tensor("cbg", [P, CB_G], f32, kind="ExternalInput").ap()
    out_d = nc.dram_tensor("out", [bpc, C, N], f32, kind="ExternalOutput").ap()

    with tile.TileContext(nc) as tc:
        with (
            tc.tile_pool(name="consts", bufs=1) as consts,
            tc.tile_pool(name="xp", bufs=1) as xp,
            tc.tile_pool(name="xnp", bufs=1) as xnp,
            tc.tile_pool(name="qkp", bufs=1) as qkp,
            tc.tile_pool(name="vp", bufs=1) as vp,
            tc.tile_pool(name="etp", bufs=2) as etp,
            tc.tile_pool(name="aop", bufs=2) as aop,
            tc.tile_pool(name="smp", bufs=2) as smp,
            tc.tile_pool(name="pst", bufs=2, space="PSUM") as pst,
            tc.tile_pool(name="pud", bufs=1, space="PSUM") as pud,
            tc.tile_pool(name="pqm", bufs=2, space="PSUM") as pqm,
        ):
            # ---- SBUF constants + input DMAs (spread across HWDGE queues).
            wt = consts.tile([P, FT], bf16, tag="warm")
            nc.gpsimd.memset(wt[:], 0.0)

            cbw = consts.tile([P, CB_W], MM_DT, tag="cbw")
            nc.scalar.dma_start(cbw[:], cbw_d[:])
            cbg = consts.tile([P, CB_G], f32, tag="cbg")
            nc.sync.dma_start(cbg[:], cbg_d[:])

            xbt = []
            for b in range(bpc):
                t = xp.tile([P, CT * N], bf16, tag=f"xb{b}")
                xbt.append(t)
            # critical-path tiles first, each on its own queue
            nc.sync.dma_start(xbt[0][:, 0:N], xb_d[0, 0:P, :])
            nc.gpsimd.dma_start(xbt[0][:, N : 2 * N], xb_d[0, P : 2 * P, :])
            if bpc > 1:
                nc.scalar.dma_start(xbt[1][:, 0:N], xb_d[1, 0:P, :])
                nc.scalar.dma_start(xbt[1][:, N : 2 * N], xb_d[1, P : 2 * P, :])

            def w_ap(i, kt):  # (128, C) lhsT slice of weight i, k-tile kt
                base = OFF_W + i * (CT * C) + kt * C
                return cbw[:, base : base + C]

            ones_ap = cbw[:, OFF_ONES : OFF_ONES + P]
            id_ap = cbw[:, OFF_ID : OFF_ID + P]
            gw = [cbg[:, OFF_GNWB + ct * 2 : OFF_GNWB + (ct + 1) * 2] for ct in range(CT)]
            gm = [cbg[:, OFF_GMASK + ct * G : OFF_GMASK + (ct + 1) * G] for ct in range(CT)]
            gmt = [cbg[0:G, OFF_GMT + ct * P : OFF_GMT + (ct + 1) * P] for ct in range(CT)]
            eps_ap = cbg[0:G, OFF_EPS : OFF_EPS + 1]
            WQ, WK, WV, WP_ = 0, 1, 2, 3

            # ---- warmup MMs (cold): trip the HAM clock gate.
            wps1 = pst.tile([P, FT], f32, tag="st")
            for _ in range(N_WARM1):
                nc.tensor.matmul(wps1[:], wt[:, 0:P], wt[:], start=True, stop=True)

            # ---- GroupNorm --------------------------------------------------
            s12_all = {}

            def gn_stats(b):
                """DVE-only stats: s1 via reduce, s2 via fused mul+reduce."""
                s12s = []
                for ct in range(CT):
                    xsl = xbt[b][:, ct * N : (ct + 1) * N]
                    s12 = smp.tile([P, 2], f32, tag=f"s12_{ct}")
                    nc.vector.reduce_sum(s12[:, 0:1], xsl, AX.X)
                    sq = smp.tile([P, N], f32, tag="sq")
                    nc.scalar.activation(sq[:], xsl, AF.Square, accum_out=s12[:, 1:2])
                    s12s.append(s12)
                s12_all[b] = s12s

            def gn_mm1(b):
                gstats = pqm.tile([G, 2], f32, tag="qm")
                for ct in range(CT):
                    nc.tensor.matmul(
                        gstats[:], gm[ct], s12_all[b][ct][:],
                        start=(ct == 0), stop=(ct == CT - 1),
                    )
                return gstats

            def gn_chain_pre(b, gstats):
                """DVE/ACT: rstd + mean from group stats."""
                mrs = smp.tile([G, 2], f32, tag="mrs")  # col0 = rstd, col1 = mean
                nc.vector.tensor_copy(mrs[:, 1:2], gstats[:, 0:1])
                negvar = smp.tile([G, 1], f32, tag="negvar")
                nc.vector.scalar_tensor_tensor(
                    negvar[:], mrs[:, 1:2], mrs[:, 1:2], gstats[:, 1:2],
                    ALU.mult, ALU.subtract,
                )
                std = smp.tile([G, 1], f32, tag="std")
                nc.scalar.activation(std[:], negvar[:], AF.Sqrt, bias=eps_ap, scale=-1.0)
                nc.vector.reciprocal(mrs[:, 0:1], std[:])
                return mrs

            def gn_mm2(b, mrs, ct):
                bc = pqm.tile([P, 2], f32, tag="qm")
                nc.tensor.matmul(bc[:], gmt[ct], mrs[:], start=True, stop=True)
                return bc

            def gn_sbias(b, bc, ct):
                scale = smp.tile([P, 1], f32, tag=f"scale{ct}")
                nc.vector.tensor_tensor(scale[:], bc[:, 0:1], gw[ct][:, 0:1], ALU.mult)
                nbias = smp.tile([P, 1], f32, tag=f"nbias{ct}")
                nc.vector.tensor_tensor(nbias[:], bc[:, 1:2], scale[:], ALU.mult)
                nc.vector.tensor_tensor(nbias[:], gw[ct][:, 1:2], nbias[:], ALU.subtract)
                return scale, nbias

            def gn_xn(b, ct, scale, nbias, xn_t):
                # xn = xb*scale + nbias, per-partition affine on DVE
                for nt in range(2):
                    sl = slice(ct * N + nt * FT, ct * N + (nt + 1) * FT)
                    nc.vector.tensor_scalar(
                        xn_t[:, sl], xbt[b][:, sl], scale[:], nbias[:],
                        ALU.mult, ALU.add,
                    )

            xn_all = {}

            def gn_front(b, xn_t):
                """Everything except the PE matmuls for batch b's GN."""
                gstats = gn_mm1(b)
                mrs = gn_chain_pre(b, gstats)
                for ct in range(CT):
                    bc = gn_mm2(b, mrs, ct)
                    sc_, nb_ = gn_sbias(b, bc, ct)
                    gn_xn(b, ct, sc_, nb_, xn_t)

            gn_stats(0)
            xn_all[0] = xnp.tile([P, CT * N], bf16, tag="xn0", name="xn0")
            gn_front(0, xn_all[0])

            # bridge warmup: keep PE busy while ACT computes xn
            wps2 = pst.tile([P, FT], f32, tag="st")
            for _ in range(N_WARM2):
                nc.tensor.matmul(wps2[:], wt[:, 0:P], wt[:], start=True, stop=True)

            if bpc > 1:
                gn_stats(1)
                xn_all[1] = xnp.tile([P, CT * N], bf16, tag="xn1", name="xn1")
                gn_front(1, xn_all[1])

            # ---- QKV -------------------------------------------------------
            q_t, k_t, vT = {}, {}, {}

            def qk_chunks(b):
                """4 chunks: q[ot], k[ot] as (128,1024) psum + DVE cast."""
                q_t[b] = [qkp.tile([P, N], MM_DT, tag=f"q{b}{ot}", name=f"q{b}{ot}") for ot in range(CT)]
                k_t[b] = [qkp.tile([P, N], MM_DT, tag=f"k{b}{ot}", name=f"k{b}{ot}") for ot in range(CT)]
                chunks = []
                for wi, dst in ((WQ, q_t[b]), (WK, k_t[b])):
                    for ot in range(CT):
                        def c(wi=wi, ot=ot, dst=dst):
                            ps = pst.tile([P, N], f32, tag="st")
                            for nt in range(2):
                                sl = slice(nt * FT, (nt + 1) * FT)
                                for kt in range(CT):
                                    nc.tensor.matmul(
                                        ps[:, sl],
                                        w_ap(wi, kt)[:, ot * P : (ot + 1) * P],
                                        xn_all[b][:, kt * N + nt * FT : kt * N + (nt + 1) * FT],
                                        start=(kt == 0), stop=(kt == CT - 1),
                                    )
                            nc.vector.tensor_copy(dst[ot][:], ps[:])
                        chunks.append(c)
                return chunks

            def v_chunks(b):
                """4 chunks of 2 mt each: vT tiles via xn^T @ Wv."""
                vT[b] = vp.tile([P, JT * C], MM_DT, tag=f"vt{b}", name=f"vt{b}")
                chunks = []
                for mt0 in range(0, JT, 2):
                    def c(mt0=mt0, b=b):
                        for mt in (mt0, mt0 + 1):
                            ps = pqm.tile([P, C], f32, tag="qm")
                            for kt in range(CT):
                                nc.tensor.matmul(
                                    ps[:],
                                    xn_all[b][:, kt * N + mt * P : kt * N + (mt + 1) * P],
                                    w_ap(WV, kt),
                                    start=(kt == 0), stop=(kt == CT - 1),
                                )
                            nc.vector.tensor_copy(vT[b][:, mt * C : (mt + 1) * C], ps[:])
                    chunks.append(c)
                return chunks

            # ---- attention pieces ------------------------------------------
            def sc_items(b, h, et_tile):
                items = []
                for jt in range(JT):
                    def s(jt=jt, b=b, h=h, et_tile=et_tile):
                        st = pst.tile([P, N], f32, tag="st")
                        for nt in range(2):
                            sl = slice(nt * FT, (nt + 1) * FT)
                            nc.tensor.matmul(
                                st[:, sl],
                                k_t[b][h][:, jt * P : (jt + 1) * P],
                                q_t[b][h][:, sl],
                                start=True, stop=True,
                            )
                        nc.scalar.activation(
                            et_tile[:, jt * N : (jt + 1) * N], st[:],
                            AF.Exp, scale=ATT_SCALE,
                        )
                    items.append(s)
                return items

            ao_t = {}

            def du_chunks(b, h, et_tile):
                """per half: 8 jt chunks (d MM + u MM) + 1 ao chunk (DVE)."""
                if (b, h) not in ao_t:
                    ao_t[(b, h)] = aop.tile([P, N], MM_DT, tag=f"ao{h}", name=f"ao{b}{h}")
                ao = ao_t[(b, h)]
                ud = {}
                chunks = []
                for half in range(2):
                    for jt in range(JT):
                        def c(half=half, jt=jt, b=b, h=h, et_tile=et_tile):
                            if jt == 0:
                                ud[half] = (
                                    pud.tile([P, FT], f32, tag="u", name="u"),
                                    pud.tile([P, FT], f32, tag="d", name="d"),
                                )
                            u_, d_ = ud[half]
                            sl = et_tile[:, jt * N + half * FT : jt * N + half * FT + FT]
                            nc.tensor.matmul(
                                d_[:], ones_ap, sl, start=(jt == 0), stop=(jt == JT - 1)
                            )
                            nc.tensor.matmul(
                                u_[:],
                                vT[b][:, jt * C + h * HD : jt * C + h * HD + HD],
                                sl,
                                start=(jt == 0), stop=(jt == JT - 1),
                            )
                        chunks.append(c)
                    def ao_c(half=half):
                        u_, d_ = ud[half]
                        r = smp.tile([P, FT], f32, tag="r")
                        nc.vector.reciprocal_approx_fast(out=r[:], in_=d_[:])
                        nc.vector.tensor_tensor(
                            ao[:, half * FT : (half + 1) * FT], u_[:], r[:], ALU.mult
                        )
                    chunks.append(ao_c)
                return chunks

            def proj_chunks(b):
                """4 chunks (ot, nt): 3 accumulating MMs + PSUM->DRAM DMA."""
                engs = [nc.sync, nc.gpsimd] if b == 0 else [nc.sync, nc.scalar]
                chunks = []
                for ot in range(CT):
                    for nt in range(2):
                        def c(ot=ot, nt=nt, b=b):
                            pj = pqm.tile([P, FT], f32, tag="qm")
                            sl = slice(nt * FT, (nt + 1) * FT)
                            for hh in range(HEADS):
                                nc.tensor.matmul(
                                    pj[:],
                                    w_ap(WP_, hh)[:, ot * P : (ot + 1) * P],
                                    ao_t[(b, hh)][:, sl],
                                    start=(hh == 0), stop=False,
                                )
                            nc.tensor.matmul(
                                pj[:], id_ap,
                                xbt[b][:, ot * N + nt * FT : ot * N + (nt + 1) * FT],
                                start=False, stop=True,
                            )
                            ot_sb = smp.tile([P, FT], f32, tag="osb", name="osb")
                            nc.vector.tensor_copy(ot_sb[:], pj[:])
                            engs[(ot * 2 + nt) % 2].dma_start(
                                out_d[b, ot * P : (ot + 1) * P, sl], ot_sb[:]
                            )
                        chunks.append(c)
                return chunks

            def weave(score_it, fillers):
                fi = 0
                ns = max(1, len(score_it))
                for i, s in enumerate(score_it):
                    s()
                    target = (i + 1) * len(fillers) // ns
                    while fi < target:
                        fillers[fi]()
                        fi += 1
                while fi < len(fillers):
                    fillers[fi]()
                    fi += 1

            # ---- global schedule -------------------------------------------
            et = {}
            # P1: qkv b0
            for c in qk_chunks(0):
                c()
            for c in v_chunks(0):
                c()
            # P2: scores b0h0 woven with qkv b1
            et[(0, 0)] = etp.tile([P, JT * N], MM_DT, tag="et", name="et00")
            fill_p2 = (qk_chunks(1) + v_chunks(1)) if bpc > 1 else []
            weave(sc_items(0, 0, et[(0, 0)]), fill_p2)
            # P3: scores b0h1 woven with du b0h0
            et[(0, 1)] = etp.tile([P, JT * N], MM_DT, tag="et", name="et01")
            weave(sc_items(0, 1, et[(0, 1)]), du_chunks(0, 0, et[(0, 0)]))
            if bpc > 1:
                # P4: scores b1h0 woven with du b0h1
                et[(1, 0)] = etp.tile([P, JT * N], MM_DT, tag="et", name="et10")
                weave(sc_items(1, 0, et[(1, 0)]), du_chunks(0, 1, et[(0, 1)]))
                # P5: scores b1h1 woven with proj b0 + du b1h0
                et[(1, 1)] = etp.tile([P, JT * N], MM_DT, tag="et", name="et11")
                weave(
                    sc_items(1, 1, et[(1, 1)]),
                    proj_chunks(0) + du_chunks(1, 0, et[(1, 0)]),
                )
                # P6: drain du b1h1 + proj b1
                for c in du_chunks(1, 1, et[(1, 1)]):
                    c()
                for c in proj_chunks(1):
                    c()
            else:
                for c in du_chunks(0, 1, et[(0, 1)]):
                    c()
                for c in proj_chunks(0):
                    c()

    nc.compile()
    return nc


def build_const_blob(gn_w, gn_b, wq, wk, wv, wp):
    """Returns (cbw bf16 [P, CB_W], cbg f32 [P, CB_G])."""
    import ml_dtypes

    cbw = np.zeros((P, CB_W), np.float32)
    for i, wmat in enumerate((wq, wk, wv, wp)):
        wT = np.asarray(wmat, np.float32).T  # (c_in, c_out)
        for kt in range(CT):
            cbw[:, OFF_W + i * CT * C + kt * C : OFF_W + i * CT * C + (kt + 1) * C] = (
                wT[kt * P : (kt + 1) * P, :]
            )
    cbw[:, OFF_ONES : OFF_ONES + P] = 1.0
    cbw[:, OFF_ID : OFF_ID + P] = np.eye(P, dtype=np.float32)
    cbg = np.zeros((P, CB_G), np.float32)
    cbg[:, OFF_GNWB + 0 : OFF_GNWB + 4 : 2] = np.asarray(gn_w, np.float32).reshape(CT, P).T
    cbg[:, OFF_GNWB + 1 : OFF_GNWB + 4 : 2] = np.asarray(gn_b, np.float32).reshape(CT, P).T
    for ct in range(CT):
        for p in range(P):
            g = (ct * P + p) // GSIZE
            cbg[p, OFF_GMASK + ct * G + g] = 1.0 / NG
            cbg[g, OFF_GMT + ct * P + p] = 1.0
    cbg[0:G, OFF_EPS] = EPS
    return cbw.astype(ml_dtypes.bfloat16), cbg


_NC_CACHE = {}


def kernel(x, gn_w, gn_b, wq, wk, wv, wp):
    import ml_dtypes

    x = np.ascontiguousarray(np.asarray(x, dtype=np.float32))
    b, c, h, w = x.shape
    xrb = x.reshape(b, c, h * w).astype(ml_dtypes.bfloat16)
    cbw, cbg = build_const_blob(gn_w, gn_b, wq, wk, wv, wp)

    if "nc" not in _NC_CACHE:
        _NC_CACHE["nc"] = build_bass()
    nc = _NC_CACHE["nc"]

    in_maps = [
        dict(
            xb=np.ascontiguousarray(xrb[i * BPC : (i + 1) * BPC]),
            cbw=cbw,
            cbg=cbg,
        )
        for i in range(N_CORES)
    ]
    res = run_bass_kernel_spmd(nc, in_maps, list(range(N_CORES)))
    out = np.concatenate([res.results[i]["out"] for i in range(N_CORES)], axis=0)
    return out.reshape(b, c, h, w).astype(np.float32)


if __name__ == "__main__":
    rng = np.random.default_rng(0)
    ins = {
        "x": rng.standard_normal((B, C, H, W), dtype=np.float32),
        "gn_w": np.ones((C,), np.float32),
        "gn_b": np.zeros((C,), np.float32),
        "wq": rng.standard_normal((C, C), dtype=np.float32) * C**-0.5,
        "wk": rng.standard_normal((C, C), dtype=np.float32) * C**-0.5,
        "wv": rng.standard_normal((C, C), dtype=np.float32) * C**-0.5,
        "wp": rng.standard_normal((C, C), dtype=np.float32) * C**-0.5,
    }
    out = kernel(**ins)
    print(out.shape, out.dtype)
